# revision 12
# baseline (speedup 1.0000x reference)
"""Trainium2 Bass kernel for nn_MultiHeadAttentionQuantum.

Reference computation (per batch element b, sharded batch-parallel over 8 cores):
    q[s, h, w]  = x[s, 128]   split into 16 heads x 8 wires
    c           = cos(q + theta[w])            (theta broadcast over wires)
    cp          = cumprod(c, axis=w)           per 8-wire group
    out[s,h,0]  = prod(c[s,h,1:8]);  out[s,h,w>=1] = cp[s,h,w]
    O = out merged back to [S=2048, E=128]
    scores = O @ O.T / sqrt(8)                 (symmetric!)
    attn   = softmax(scores, axis=-1)
    y      = attn @ O

Device strategy per core (one batch element each):
  - layout: [128 partitions = s%128, free = (n=s//128, e)]
  - sin via ACT table (|arg|<4 domain) after magic-number range reduction
  - segmented cumprod via 13 strided DVE multiplies
  - scores via split-TF32 matmul: O = H + L (fp32r high/low parts);
    O@O.T ~= H@H.T + H@L.T + L@H.T  -> fp32-grade accuracy at 1cyc/row
  - softmax without row-max: scores <= 128/sqrt(8) = C, use exp(s/sqrt(8) - C);
    row sums fused into the exp via ACT accum_out
  - attn @ O via symmetry of E=exp(scores): E row-block kt doubles as the
    column block needed for the second matmul's moving operand (fp32r, N=512)
  - output produced as yT [e, s] chunks, PE-transposed back and scaled by
    1/rowsum during the PSUM->SBUF copy
"""

import math
from contextlib import ExitStack

import numpy as np

import concourse.bass as bass
import concourse.tile as tile
from concourse import bacc, mybir
from concourse.bass_utils import run_bass_kernel_spmd
from concourse.masks import make_identity

B = 8          # batch -> one per core
S = 2048       # sequence length
E = 128        # embed dim
NB = S // 128  # 16 row blocks
W = 8          # wires per head
NH = E // W    # 16 heads

F32 = mybir.dt.float32
F32R = mybir.dt.float32r

TWO_PI = float(2 * np.pi)
INV_TWO_PI = float(1 / (2 * np.pi))
MAGIC = float(1.5 * 2**23)          # fp32 round-to-nearest-int trick
HALF_PI = float(np.pi / 2)
INV_SQRT8 = float(1 / math.sqrt(8))
SCORE_MAX = float(E / math.sqrt(8))  # upper bound on any score


def build_kernel(n_cores: int = B):
    nc = bacc.Bacc(
        trn_type="TRN2", target_bir_lowering=False, debug=False,
        num_devices=n_cores,
    )
    x = nc.dram_tensor("x", [S, E], F32, kind="ExternalInput")
    theta = nc.dram_tensor("theta", [E], F32, kind="ExternalInput")
    y = nc.dram_tensor("y", [S, E], F32, kind="ExternalOutput")

    with tile.TileContext(nc) as tc, ExitStack() as ctx:
        pq = ctx.enter_context(tc.tile_pool(name="pq", bufs=3))
        pr = ctx.enter_context(tc.tile_pool(name="pr", bufs=1))
        pE = ctx.enter_context(tc.tile_pool(name="pE", bufs=NB))
        psmall = ctx.enter_context(tc.tile_pool(name="psmall", bufs=1))
        pstage = ctx.enter_context(tc.tile_pool(name="pstage", bufs=2))
        py = ctx.enter_context(tc.tile_pool(name="py", bufs=2))
        ptrans = ctx.enter_context(tc.tile_pool(name="ptrans", bufs=2, space="PSUM"))
        pscore = ctx.enter_context(tc.tile_pool(name="pscore", bufs=2, space="PSUM"))
        pout2 = ctx.enter_context(tc.tile_pool(name="pout2", bufs=2, space="PSUM"))

        ident = psmall.tile([128, 128], F32)
        make_identity(nc, ident)
        neg_cmax = psmall.tile([128, 1], F32)
        nc.vector.memset(neg_cmax, -SCORE_MAX)

        # ---- load x as [p=s%128, (n, e)] and theta broadcast over partitions
        xt = pq.tile([128, S], F32, tag="big")
        nc.sync.dma_start(
            out=xt.rearrange("p (n e) -> p n e", e=E),
            in_=x.ap().rearrange("(n p) e -> p n e", p=128),
        )
        th = psmall.tile([128, E], F32)
        th_src = theta.ap()
        nc.sync.dma_start(
            out=th,
            in_=bass.AP(tensor=th_src.tensor, offset=th_src.offset,
                        ap=[[0, 128]] + list(th_src.ap)),
        )

        # ---- a = (x + pi/2) + theta   (cos(z) = sin(z + pi/2))
        # theta broadcast along the n free dim via a step-0 AP
        th_b = bass.AP(tensor=th.tensor, offset=th.offset,
                       ap=[list(th.ap[0]), [0, NB], list(th.ap[1])])
        nc.vector.scalar_tensor_tensor(
            out=xt.rearrange("p (n e) -> p n e", e=E),
            in0=xt.rearrange("p (n e) -> p n e", e=E),
            scalar=HALF_PI, in1=th_b,
            op0=mybir.AluOpType.add, op1=mybir.AluOpType.add,
        )
        # ---- range-reduce a to [-pi, pi]: a -= 2pi * round(a / 2pi)
        k = pq.tile([128, S], F32, tag="big")
        nc.vector.tensor_scalar(
            out=k, in0=xt, scalar1=INV_TWO_PI, scalar2=MAGIC,
            op0=mybir.AluOpType.mult, op1=mybir.AluOpType.add,
        )
        nc.vector.tensor_scalar(
            out=k, in0=k, scalar1=MAGIC, scalar2=-TWO_PI,
            op0=mybir.AluOpType.subtract, op1=mybir.AluOpType.mult,
        )
        nc.vector.tensor_add(out=xt, in0=xt, in1=k)
        # ---- c = sin(a)
        c = pq.tile([128, S], F32, tag="big")
        nc.scalar.activation(out=c, in_=xt, func=mybir.ActivationFunctionType.Sin)

        # ---- segmented cumprod over wires within each head
        c4 = c.rearrange("p (n h w) -> p n h w", h=NH, w=W)
        O = pq.tile([128, S], F32, tag="big")
        O4 = O.rearrange("p (n h w) -> p n h w", h=NH, w=W)
        # O[..., w>=1] = cumprod(c)[..., w]
        nc.vector.tensor_mul(out=O4[:, :, :, 1], in0=c4[:, :, :, 0], in1=c4[:, :, :, 1])
        for w in range(2, W):
            nc.vector.tensor_mul(
                out=O4[:, :, :, w], in0=O4[:, :, :, w - 1], in1=c4[:, :, :, w]
            )
        # O[..., 0] = prod(c[..., 1:8])
        nc.vector.tensor_mul(out=O4[:, :, :, 0], in0=c4[:, :, :, 1], in1=c4[:, :, :, 2])
        for w in range(3, W):
            nc.vector.tensor_mul(
                out=O4[:, :, :, 0], in0=O4[:, :, :, 0], in1=c4[:, :, :, w]
            )

        # ---- split into TF32 high part (natural layout, for attn@O)
        H = pr.tile([128, S], F32R)
        nc.vector.tensor_copy(out=H, in_=O)

        # ---- transpose O -> [e, s] blocks, split each into HT/LT fp32r parts
        HT = pr.tile([128, S], F32R)
        LT = pr.tile([128, S], F32R)
        for nb in range(NB):
            blk = slice(nb * 128, (nb + 1) * 128)
            pt = ptrans.tile([128, 128], F32)
            nc.tensor.transpose(out=pt, in_=O[:, blk], identity=ident)
            nc.vector.tensor_copy(out=HT[:, blk], in_=pt)
            nc.vector.tensor_sub(out=LT[:, blk], in0=pt, in1=HT[:, blk])

        # ---- scores + exp, one 128-row block x 1024-col half at a time
        r_all = psmall.tile([128, 2 * NB], F32)
        E_tiles = []
        for i in range(NB):
            Ei = pE.tile([128, S], F32R)
            E_tiles.append(Ei)
            for hf in range(2):
                ps = pscore.tile([128, 1024], F32)
                for term, (lhs, rhs) in enumerate(((HT, HT), (HT, LT), (LT, HT))):
                    for cc in range(2):
                        col = hf * 1024 + cc * 512
                        nc.tensor.matmul(
                            out=ps[:, cc * 512:(cc + 1) * 512],
                            lhsT=lhs[:, i * 128:(i + 1) * 128],
                            rhs=rhs[:, col:col + 512],
                            start=(term == 0), stop=(term == 2),
                        )
                nc.scalar.activation(
                    out=Ei[:, hf * 1024:(hf + 1) * 1024], in_=ps,
                    func=mybir.ActivationFunctionType.Exp,
                    bias=neg_cmax, scale=INV_SQRT8,
                    accum_out=r_all[:, 2 * i + hf:2 * i + hf + 1],
                )

        # ---- softmax denominators
        r = psmall.tile([128, NB], F32)
        ra = r_all.rearrange("p (i two) -> p i two", two=2)
        nc.vector.tensor_add(out=r, in0=ra[:, :, 0], in1=ra[:, :, 1])
        recip = psmall.tile([128, NB], F32)
        nc.vector.reciprocal(out=recip, in_=r)

        # ---- yT[e, s] = sum_t H[t, e] * E[t, s]  (E row-block == col-block by symmetry)
        for j in range(4):
            po = pout2.tile([128, 512], F32)
            for kt in range(NB):
                nc.tensor.matmul(
                    out=po,
                    lhsT=H[:, kt * 128:(kt + 1) * 128],
                    rhs=E_tiles[kt][:, j * 512:(j + 1) * 512],
                    start=(kt == 0), stop=(kt == NB - 1),
                )
            stage = pstage.tile([128, 512], F32)
            nc.vector.tensor_copy(out=stage, in_=po)
            for jj in range(4):
                si = j * 4 + jj
                pt = ptrans.tile([128, 128], F32)
                nc.tensor.transpose(
                    out=pt, in_=stage[:, jj * 128:(jj + 1) * 128], identity=ident
                )
                y_sb = py.tile([128, 128], F32)
                nc.vector.tensor_scalar_mul(
                    out=y_sb, in0=pt, scalar1=recip[:, si:si + 1]
                )
                nc.sync.dma_start(
                    out=y.ap().rearrange("(n p) e -> n p e", p=128)[si], in_=y_sb
                )

    nc.compile()
    return nc


_NC_CACHE = None


def _get_nc():
    global _NC_CACHE
    if _NC_CACHE is None:
        _NC_CACHE = build_kernel()
    return _NC_CACHE


def kernel(x: np.ndarray, theta: np.ndarray) -> np.ndarray:
    """x: [8, 2048, 128] f32, theta: [8] f32 -> [8, 2048, 128] f32."""
    assert x.shape == (B, S, E) and theta.shape == (W,)
    nc = _get_nc()
    theta_row = np.tile(np.ascontiguousarray(theta, dtype=np.float32), E // W)
    in_maps = [
        {"x": np.ascontiguousarray(x[b], dtype=np.float32), "theta": theta_row}
        for b in range(B)
    ]
    res = run_bass_kernel_spmd(nc, in_maps, core_ids=list(range(B)))
    return np.stack([res.results[b]["y"] for b in range(B)], axis=0)
